# revision 22
# baseline (speedup 1.0000x reference)
"""Trainium2 Bass kernel for GatedCrossAttention (B=4, N=4096, C=1024, H=16, M=4).

Reference math (dead code removed: the v/gate projections are overwritten
by views of k in the original module, so v = g = k):
    q = query @ Wq.T + bq                    [B,N,C]   -> [B,N,H,hd]
    k = key   @ Wk.T + bk                    [B,N,M,C] -> [B,N,M,H,hd]
    attn = softmax_M(SCALE * einsum('bnhc,bnmhc->bnmh', q, k))
    out  = einsum('bnmh,bnmhc->bnhc', attn, k*k) . reshape(B,N,C)
    out  = out @ Wo.T + bo

Strategy: data parallel over the 16384 tokens (8 cores x 2048), no
collectives.  Projections run token-major: the activation tile (channel-major
in SBUF, 128 channels x 128 tokens) is the *stationary* matmul operand and
the weight chunk is the moving operand, so PSUM holds [token, channel_out]
tiles.  The whole attention middle (logits = per-head dot products, softmax
over the M=4 window, weighting of k^2) then runs on contiguous free-axis
DVE ops -- no indicator matmuls, no partition broadcasts, no strided
reductions -- split into two half-head chains per tile so downstream stages
unblock early; k^2 runs on ScalarE (Square).  Only the output projection
needs channels back on partitions, done by the DMA xbar transpose engine
(dma_start_transpose), so the PE runs GEMM matmuls only: 96 per 128-token
tile (contraction 1024, moving 512), software-pipelined with a 2-tile lag.
Inputs are host-tiled for single-descriptor-per-partition DMA; dummy warmup
matmuls keep the PE clock un-throttled through the initial DMA fill and the
final drain.  Everything is bf16 on chip (rel err vs f32 reference ~6e-3);
accumulation stays f32 in PSUM.
"""

import dataclasses
import numpy as np
from contextlib import ExitStack

try:
    import concourse.bass as bass
except ImportError:  # path fallback for bare containers
    import sys

    sys.path.insert(0, "/opt/trn_rl_repo")
    import concourse.bass as bass

import concourse.tile as tile
from concourse import bacc, mybir
from concourse.bass_utils import run_bass_kernel_spmd

# problem constants (hardcoded per the task contract)
B, N, C, H, HD, M = 4, 4096, 1024, 16, 64, 4
SCALE = float(HD) ** -0.5
NCORES = 8
T_TOTAL = B * N
T_CORE = T_TOTAL // NCORES  # 2048
TILE = 128                  # tokens per compute tile (PSUM partition limit)
TB = 512                    # tokens per DMA block
NJ = C // 128               # 8 channel chunks
NT = TB // TILE             # 4 tiles per block

DT = mybir.dt.bfloat16
import ml_dtypes
NPDT = ml_dtypes.bfloat16
F32 = mybir.dt.float32

EXP = mybir.ActivationFunctionType.Exp
AXX = mybir.AxisListType.X


def _bcast(ap, reps, axis):
    """Insert a 0-stride dim of size `reps` at AP position `axis` (0=partition)."""
    new = list(ap.ap)
    new.insert(axis, [0, reps])
    return dataclasses.replace(ap, ap=new)


def build_nc(t_core=T_CORE, with_bias=False):
    ntile = t_core // TILE
    nblk = t_core // TB
    nc = bacc.Bacc("TRN2", target_bir_lowering=False, debug=False)

    # inputs host-tiled so each DMA reads one contiguous 8KB run per partition:
    # qT[b, p, j, t] = query_ct[j*128+p, b*TB+t]
    qTd = nc.declare_dram_parameter("qT", [t_core // TB, 128, NJ * TB], DT,
                                    isOutput=False)
    kTd = nc.declare_dram_parameter("kT", [M, t_core // TB, 128, NJ * TB], DT,
                                    isOutput=False)
    wqT = nc.declare_dram_parameter("wqT", [C, C], DT, isOutput=False)
    wkT = nc.declare_dram_parameter("wkT", [C, C], DT, isOutput=False)
    woT = nc.declare_dram_parameter("woT", [C, C], DT, isOutput=False)
    if with_bias:
        bq = nc.declare_dram_parameter("bq", [1, C], DT, isOutput=False)
        bk = nc.declare_dram_parameter("bk", [1, C], DT, isOutput=False)
        bo = nc.declare_dram_parameter("bo", [1, C], DT, isOutput=False)
    out = nc.declare_dram_parameter("out", [t_core, C], F32, isOutput=True)

    # DRAM views
    qT_v = qTd.ap().rearrange("b p (j t) -> b p j t", j=NJ)
    kT_v = kTd.ap().rearrange("m b p (j t) -> m b p j t", j=NJ)
    wq_v = wqT.ap().rearrange("(c p) j -> p c j", p=128)
    wk_v = wkT.ap().rearrange("(c p) j -> p c j", p=128)
    # xbar transpose writes ycm[p, j, t] = y[t, j*128+p]: standard chunking
    wo_v = woT.ap().rearrange("(c p) j -> p c j", p=128)

    with tile.TileContext(nc) as tc, ExitStack() as ctx:
        consts = ctx.enter_context(tc.tile_pool(name="consts", bufs=1))
        p_inq = ctx.enter_context(tc.tile_pool(name="inq", bufs=2))
        p_ink = ctx.enter_context(tc.tile_pool(name="ink", bufs=8))
        p_qsb = ctx.enter_context(tc.tile_pool(name="qsb", bufs=2))
        p_ksb = ctx.enter_context(tc.tile_pool(name="ksb", bufs=2))
        p_prod = ctx.enter_context(tc.tile_pool(name="prod", bufs=1))
        p_ksq = ctx.enter_context(tc.tile_pool(name="ksq", bufs=1))
        p_sm = ctx.enter_context(tc.tile_pool(name="sm", bufs=2))
        p_ct = ctx.enter_context(tc.tile_pool(name="ct", bufs=1))
        p_y = ctx.enter_context(tc.tile_pool(name="y", bufs=2))
        p_ycm = ctx.enter_context(tc.tile_pool(name="ycm", bufs=2))
        p_osb = ctx.enter_context(tc.tile_pool(name="osb", bufs=2))
        pq = ctx.enter_context(tc.tile_pool(name="pq", bufs=1, space="PSUM"))
        pk = ctx.enter_context(tc.tile_pool(name="pk", bufs=2, space="PSUM"))
        po = ctx.enter_context(tc.tile_pool(name="po", bufs=1, space="PSUM"))

        # ---- constants / weights (resident, chunked so MMs depend on one DMA) ----
        wq_js = [consts.tile([128, C], DT, tag=f"wq{j}", name=f"wq{j}")
                 for j in range(NJ)]
        wk_js = [consts.tile([128, C], DT, tag=f"wk{j}", name=f"wk{j}")
                 for j in range(NJ)]
        wo_js = [consts.tile([128, C], DT, tag=f"wo{j}", name=f"wo{j}")
                 for j in range(NJ)]
        for j in range(NJ):
            for h in range(2):
                cs = slice(h * 512, (h + 1) * 512)
                nc.sync.dma_start(out=wq_js[j][:, cs], in_=wq_v[:, j, cs])
        if with_bias:
            ones_sb = consts.tile([1, TILE], DT)
            nc.vector.memset(ones_sb, 1.0)
            bq_sb = consts.tile([1, C], DT)
            bk_sb = consts.tile([1, C], DT)
            bo_sb = consts.tile([1, C], DT)
            nc.sync.dma_start(out=bq_sb, in_=bq.ap())
            nc.sync.dma_start(out=bk_sb, in_=bk.ap())
            nc.sync.dma_start(out=bo_sb, in_=bo.ap())

        def dma_q(blk, parts=2):
            q_in = p_inq.tile([128, NJ, TB], DT, tag="qin", name="qin")
            step = NJ // parts
            for h in range(parts):
                js = slice(h * step, (h + 1) * step)
                nc.sync.dma_start(out=q_in[:, js, :], in_=qT_v[blk][:, js, :])
            return q_in

        def dma_k(blk):
            k_in = []
            for m in range(M):
                kt = p_ink.tile([128, NJ, TB], DT, tag="kin", name="kin")
                for h in range(2):
                    js = slice(h * NJ // 2, (h + 1) * NJ // 2)
                    nc.sync.dma_start(out=kt[:, js, :], in_=kT_v[m, blk][:, js, :])
                k_in.append(kt)
            return k_in

        def dma_block(blk):
            return dma_q(blk), dma_k(blk)

        # startup order: wq (above) + q block 0 gate the first matmul; then
        # wk + k block 0; wo arrives while block 0 computes.
        q0 = dma_q(0, parts=4)
        for j in range(NJ):
            nc.sync.dma_start(out=wk_js[j], in_=wk_v[:, j, :])
        k0 = dma_k(0)
        for j in range(NJ):
            nc.sync.dma_start(out=wo_js[j], in_=wo_v[:, j, :])
        cur = (q0, k0)
        nxt = None

        # PE warmup: dummy matmuls during the initial DMA fill keep the HAM
        # activity window busy so real matmuls start at full clock.
        wdum = consts.tile([128, 512], DT)
        nc.vector.memset(wdum, 0.0)
        wps = po.tile([128, 2, 512], F32, tag="po", name="wps")
        for i in range(32):
            nc.tensor.matmul(wps[:, i % 2, :], wdum[:, :128], wdum,
                             start=True, stop=True)
        prevs = []  # queue of (y, t0) tiles awaiting transpose + O-proj (lag 2)

        for t in range(ntile + 2):
            tail = None
            if len(prevs) == 2 or (t >= ntile and prevs):
                tail = prevs.pop(0)
            if t < ntile:
                blk, tt = divmod(t, NT)
                if tt == 0 and t > 0:
                    cur, nxt = nxt, None
                q_in, k_in = cur
                tsl = slice(tt * TILE, (tt + 1) * TILE)

                # ---- Q projection (token-major: activations stationary) ----
                qp = pq.tile([128, 2, 512], F32, tag="pq", name="qp")
                for j in range(NJ):
                    for hf in range(2):
                        nc.tensor.matmul(
                            qp[:, hf, :],
                            q_in[:, j, tsl],
                            wq_js[j][:, hf * 512:(hf + 1) * 512],
                            start=(j == 0),
                            stop=(j == NJ - 1 and not with_bias),
                        )
                if with_bias:
                    for hf in range(2):
                        nc.tensor.matmul(
                            qp[:, hf, :], ones_sb,
                            bq_sb[:, hf * 512:(hf + 1) * 512],
                            start=False, stop=True,
                        )
                q_sb = p_qsb.tile([128, C], DT, tag="qsb", name="qsb")
                nc.scalar.copy(
                    out=q_sb.rearrange("p (u v) -> p u v", u=2), in_=qp
                )

            if tail is not None:
                ycm, t0_pv = tail

            if t < ntile:
                # prefetch next DMA block mid-way through this one
                if tt == 2 and blk + 1 < nblk:
                    nxt = dma_block(blk + 1)

                # ---- K projection ----
                k_sb = p_ksb.tile([128, M, C], DT, tag="ksb", name="ksb")
                for m in range(M):
                    kp = pk.tile([128, 2, 512], F32, tag="pk", name="kp")
                    for j in range(NJ):
                        for hf in range(2):
                            nc.tensor.matmul(
                                kp[:, hf, :],
                                k_in[m][:, j, tsl],
                                wk_js[j][:, hf * 512:(hf + 1) * 512],
                                start=(j == 0),
                                stop=(j == NJ - 1 and not with_bias),
                            )
                    if with_bias:
                        for hf in range(2):
                            nc.tensor.matmul(
                                kp[:, hf, :], ones_sb,
                                bk_sb[:, hf * 512:(hf + 1) * 512],
                                start=False, stop=True,
                            )
                    nc.scalar.copy(
                        out=k_sb[:, m, :].rearrange("p (u v) -> p u v", u=2),
                        in_=kp,
                    )
                ksq = p_ksq.tile([128, M, C], DT, tag="ksq", name="ksq")
                for hh in range(2):
                    cs = slice(hh * 512, (hh + 1) * 512)
                    nc.scalar.square(ksq[:, :, cs], k_sb[:, :, cs])

            # ---- output projection of the tailed tile ----
            if tail is not None:
                if t == ntile + 1:
                    # keep the PE busy (HAM warm) while the final chain drains
                    wq_ = pq.tile([128, 2, 512], F32, tag="pq", name="wq_")
                    for i in range(24):
                        nc.tensor.matmul(wq_[:, i % 2, :], wdum[:, :128], wdum,
                                         start=True, stop=True)
                op = po.tile([128, 2, 512], F32, tag="po", name="op")
                for j in range(NJ):
                    for hf in range(2):
                        nc.tensor.matmul(
                            op[:, hf, :],
                            ycm[:, j, :],
                            wo_js[j][:, hf * 512:(hf + 1) * 512],
                            start=(j == 0),
                            stop=(j == NJ - 1 and not with_bias),
                        )
                if with_bias:
                    for hf in range(2):
                        nc.tensor.matmul(
                            op[:, hf, :], ones_sb,
                            bo_sb[:, hf * 512:(hf + 1) * 512],
                            start=False, stop=True,
                        )
                osb = p_osb.tile([128, C], F32, tag="osb", name="osb")
                nc.scalar.copy(
                    out=osb.rearrange("p (u v) -> p u v", u=2), in_=op
                )
                for h in range(2):
                    nc.scalar.dma_start(
                        out=out.ap()[t0_pv:t0_pv + TILE, h * 512:(h + 1) * 512],
                        in_=osb[:, h * 512:(h + 1) * 512],
                    )

            if t < ntile:
                # ---- attention middle, all free-axis ops (DVE + one exp),
                # split into two half-head chains so the downstream transpose
                # and O-proj of each half unblock as early as possible ----
                prod = p_prod.tile([128, M, C], DT, tag="prod", name="prod")
                lt = p_sm.tile([128, H, M], F32, tag="lt", name="lt")
                e = p_sm.tile([128, H, M], F32, tag="e", name="e")
                s = p_sm.tile([128, H], F32, tag="s", name="s")
                rcp = p_sm.tile([128, H], F32, tag="rcp", name="rcp")
                w = p_sm.tile([128, H, M], DT, tag="w", name="w")
                ct = p_ct.tile([128, M, C], DT, tag="ct", name="ct")
                y01 = p_y.tile([128, C], DT, tag="y01", name="y01")
                y23 = p_y.tile([128, C], DT, tag="y23", name="y23")
                y = p_y.tile([128, C], DT, tag="y", name="y", bufs=3)
                ycm = p_ycm.tile([128, NJ, TILE], DT, tag="ycm", name="ycm",
                                 bufs=3)
                HH = H // 2
                for hh in range(2):
                    cs = slice(hh * 512, (hh + 1) * 512)
                    hs = slice(hh * HH, (hh + 1) * HH)
                    nc.vector.tensor_mul(
                        prod[:, :, cs], _bcast(q_sb[:, cs], M, 1), k_sb[:, :, cs]
                    )
                    nc.vector.reduce_sum(
                        lt[:, hs, :].rearrange("p h m -> p m h"),
                        prod[:, :, cs].rearrange("p m (h x) -> p m h x", h=HH),
                        axis=AXX,
                    )
                    nc.scalar.activation(e[:, hs, :], lt[:, hs, :], func=EXP,
                                         scale=SCALE)
                    nc.vector.reduce_sum(s[:, hs], e[:, hs, :], axis=AXX)
                    nc.vector.reciprocal(rcp[:, hs], s[:, hs])
                    nc.vector.tensor_mul(w[:, hs, :], e[:, hs, :],
                                         _bcast(rcp[:, hs], M, 2))
                    nc.vector.tensor_mul(
                        ct[:, :, cs].rearrange("p m (h x) -> p m h x", h=HH),
                        ksq[:, :, cs].rearrange("p m (h x) -> p m h x", h=HH),
                        _bcast(w[:, hs, :].rearrange("p h m -> p m h"), HD, 3),
                    )
                    nc.vector.tensor_add(y01[:, cs], ct[:, 0, cs], ct[:, 1, cs])
                    nc.vector.tensor_add(y23[:, cs], ct[:, 2, cs], ct[:, 3, cs])
                    nc.vector.tensor_add(y[:, cs], y01[:, cs], y23[:, cs])
                    nc.sync.dma_start_transpose(
                        ycm[:, 4 * hh:4 * hh + 4, :], y[:, cs]
                    )
                prevs.append((ycm, t * TILE))

    nc.compile()
    return nc


def _host_prep(query, key, Wq, Wk, Wo, bq, bk, bo):
    nblk = T_CORE // TB
    # qT[core][b, p, j*TB+t] = query_ct[j*128+p, core*T_CORE + b*TB+t]
    qT = (np.asarray(query, np.float32).reshape(T_TOTAL, C).T
          .reshape(NJ, 128, NCORES, nblk, TB)
          .transpose(2, 3, 1, 0, 4)
          .reshape(NCORES, nblk, 128, NJ * TB)).astype(NPDT)
    kT = (np.asarray(key, np.float32).reshape(T_TOTAL, M, C).transpose(1, 2, 0)
          .reshape(M, NJ, 128, NCORES, nblk, TB)
          .transpose(3, 0, 4, 2, 1, 5)
          .reshape(NCORES, M, nblk, 128, NJ * TB)).astype(NPDT)

    wqT = np.ascontiguousarray(Wq.T).astype(NPDT)
    wkT = np.ascontiguousarray(Wk.T).astype(NPDT)
    woT = np.ascontiguousarray(Wo.T).astype(NPDT)

    with_bias = bool(np.any(bq) or np.any(bk) or np.any(bo))
    common = {"wqT": wqT, "wkT": wkT, "woT": woT}
    if with_bias:
        common |= {
            "bq": bq.reshape(1, C).astype(NPDT),
            "bk": bk.reshape(1, C).astype(NPDT),
            "bo": bo.reshape(1, C).astype(NPDT),
        }
    in_maps = []
    for i in range(NCORES):
        in_maps.append(
            {
                "qT": np.ascontiguousarray(qT[i]),
                "kT": np.ascontiguousarray(kT[i]),
                **common,
            }
        )
    return in_maps, with_bias


_NC_CACHE = {}
_LAST_RESULT = None


def kernel(query, key, gate, Wq, bq, Wk, bk, Wv, bv, Wg, bg, Wo, bo):
    in_maps, with_bias = _host_prep(query, key, Wq, Wk, Wo, bq, bk, bo)
    key_ = (T_CORE, with_bias)
    if key_ not in _NC_CACHE:
        _NC_CACHE[key_] = build_nc(T_CORE, with_bias)
    nc = _NC_CACHE[key_]
    res = run_bass_kernel_spmd(nc, in_maps, list(range(NCORES)))
    global _LAST_RESULT
    _LAST_RESULT = res
    out = np.concatenate([res.results[i]["out"] for i in range(NCORES)], axis=0)
    return out.reshape(B, N, C)
